# revision 29
# baseline (speedup 1.0000x reference)
"""Multi-head self-attention Trainium2 kernel (8 NeuronCores, SPMD).

Problem: B=2, N=4096, D=512, H=8 heads of dim 64.
  qkv = x @ qkv_w.T + qkv_b ; per-head attention with softmax(QK^T/8) ;
  out = attn @ out_w.T + out_b

Sharding: 16 (batch, head) pairs -> 8 cores, each core owns one batch b and
one head-PAIR (2 adjacent heads = a 128-row slice of the qkv projections).
Each core computes the full attention for its 2 heads over all 4096 rows and
a partial output projection; the host sums the 4 per-batch partials and adds
the (folded) biases.

On-chip layout strategy: everything is computed with the contraction dim on
partitions so no transposes are ever needed:
  Q^T,K^T [128d, 4096]  <- lhsT=W^T tiles, rhs=x^T
  V       [4096, 128d]  (natural; lhsT=x^T tile, rhs=Wv^T) + fused ones column
  S^T = K^T-stationary matmul, 2 heads row-packed (K=64 each) in the PE array
  P^T = exp(S^T) on ScalarE straight out of PSUM (no max-subtraction: |S|<~3)
  O^T accum = (V|1)-stationary matmul over P^T; row 64 = softmax denominator
  normalize via reciprocal + PE outer-product broadcast; partial y^T = Wout^T
  slice-stationary matmul.
Scale 1/sqrt(64) and all biases are folded on the host (wq*=0.125 etc.).

The steady state is ScalarE-exp-saturated (256 x [128,1024] Exp at ~1.11us
back-to-back = 285us); the schedule exists to keep that chain gapless:
  - PE warmup junk matmuls during the initial weight/x DMA window (HAM
    un-throttle) + minimal K projection (one 128-col quarter) before the
    first score pair, so exp #0 starts ~7us earlier.
  - K/V projections are flattened to one quantum per iteration across ic=0
    (kproj quarter at iter jt covers the jt+1 key tile just in time).
  - Per iteration the emission order is [score-pair; independent extras;
    PV(prev)], so the exp-dependent PV pair never head-of-line-blocks the
    next score pair in the PE FIFO.
  - finalize: reciprocal_approx_fast (~5x vs iterative reciprocal), and the
    pipeline-drain tail uses N=512 quanta with evictions/DMAs spread across
    the otherwise-idle Scalar/Vector engines and all four DMA queues.
"""

import os
import numpy as np
import ml_dtypes

B, N, D, H, HD = 2, 4096, 512, 8, 64
NCORES = 8
KT_TILES = 4      # D / 128 contraction tiles
JT = 32           # N / 128 key tiles
ICH = 8           # N / 512 query chunks
P = 128

# compute dtype mode: "bf16" (fast), "mixed" (fp32 scores, bf16 PV),
# "fp32" (all fp32)
MODE = os.environ.get("ATTN_KERNEL_MODE", "bf16")

_BUILD_CACHE = {}


def _np_dt(dt):
    import concourse.mybir as mybir
    return np.dtype(ml_dtypes.bfloat16) if dt == mybir.dt.bfloat16 else np.dtype(np.float32)


def _build(mode):
    """Build (and cache) the compiled Bass program for all cores (SPMD)."""
    if mode in _BUILD_CACHE:
        return _BUILD_CACHE[mode]

    import concourse.bacc as bacc
    import concourse.mybir as mybir
    import concourse.tile as tile
    from concourse.bass import _add_dep_helper
    from contextlib import ExitStack

    f32 = mybir.dt.float32
    bf16 = mybir.dt.bfloat16
    if mode == "bf16":
        dt_qk, dt_pv = bf16, bf16
    elif mode == "mixed":
        dt_qk, dt_pv = f32, bf16
    else:
        dt_qk, dt_pv = f32, f32

    Exp = mybir.ActivationFunctionType.Exp

    nc = bacc.Bacc(None, target_bir_lowering=False)
    xt_d = nc.dram_tensor("xt", [KT_TILES, P, N], dt_qk, kind="ExternalInput")
    wqt_d = nc.dram_tensor("wqt", [KT_TILES, P, P], dt_qk, kind="ExternalInput")
    wkt_d = nc.dram_tensor("wkt", [KT_TILES, P, P], dt_qk, kind="ExternalInput")
    wvt_d = nc.dram_tensor("wvt", [KT_TILES, P, P], dt_qk, kind="ExternalInput")
    wot_d = nc.dram_tensor("wot", [2, HD, D], dt_pv, kind="ExternalInput")
    bq_d = nc.dram_tensor("bq", [P, 1], f32, kind="ExternalInput")
    bk_d = nc.dram_tensor("bk", [P, 1], f32, kind="ExternalInput")
    yp_d = nc.dram_tensor("yp", [KT_TILES, P, N], f32, kind="ExternalOutput")

    def ics(i):
        return slice(i * 512, (i + 1) * 512)

    def jts(j):
        return slice(j * P, (j + 1) * P)

    def mts(m):
        return slice(m * P, (m + 1) * P)

    with tile.TileContext(nc) as tc, ExitStack() as ctx:
        const = ctx.enter_context(tc.tile_pool(name="const", bufs=1))
        sp = ctx.enter_context(tc.tile_pool(name="spool", bufs=2, space="PSUM"))
        op = ctx.enter_context(tc.tile_pool(name="opool", bufs=3, space="PSUM"))
        mp = ctx.enter_context(tc.tile_pool(name="mpool", bufs=1, space="PSUM"))
        pp = ctx.enter_context(tc.tile_pool(name="ppool", bufs=6))
        yep = ctx.enter_context(tc.tile_pool(name="yepool", bufs=3))
        rrp = ctx.enter_context(tc.tile_pool(name="rrpool", bufs=2))
        rbp = ctx.enter_context(tc.tile_pool(name="rbpool", bufs=2))

        xt = const.tile([P, KT_TILES, N], dt_qk, tag="xt")
        wqt = const.tile([P, KT_TILES, P], dt_qk, tag="wqt")
        wkt = const.tile([P, KT_TILES, P], dt_qk, tag="wkt")
        wvt = const.tile([P, KT_TILES, P], dt_qk, tag="wvt")
        # Queue assignment tuned for the prologue critical chain
        # (qproj -> kproj -> first score pair -> first exp):
        #  - sync (fast HW queue): wqt interleaved with x^T chunk 0 so the
        #    Q projection's weights AND rhs land first, then the remaining
        #    x chunks in column-major order (early kproj/vproj can start
        #    while the bulk of x streams in).
        #  - gpsimd (slow SW queue): only wkt (needed ~4us in).
        #  - scalar (shares the ScalarE FIFO with the exps -- keep short):
        #    bq/bk first (bias adds gate the first score pair), then the
        #    late-needed wvt/wot.
        bq = const.tile([P, 1], f32, tag="bq")
        bk = const.tile([P, 1], f32, tag="bk")
        nc.scalar.dma_start(bq[:], bq_d[:])
        nc.scalar.dma_start(bk[:], bk_d[:])
        for k in range(KT_TILES):
            nc.sync.dma_start(wqt[:, k, :], wqt_d[k])
        for k in range(KT_TILES):
            nc.gpsimd.dma_start(wkt[:, k, :], wkt_d[k])
        for k in range(KT_TILES):
            nc.scalar.dma_start(wvt[:, k, :], wvt_d[k])
        # x^T in column-major chunk order: the first column blocks (all
        # k-tiles) land right after wqt, so the Q/K projections and early
        # attention start ~7us sooner than waiting for whole k-tiles
        XCH = 512
        for c in range(N // XCH):
            for k in range(KT_TILES):
                nc.sync.dma_start(xt[:, k, c * XCH:(c + 1) * XCH],
                                  xt_d[k][:, c * XCH:(c + 1) * XCH])
        wot = const.tile([HD, 2, D], dt_pv, tag="wot")
        for h in range(2):
            nc.scalar.dma_start(wot[:, h, :], wot_d[h])

        QT = const.tile([P, N], dt_qk, tag="QT")
        KT = const.tile([P, N], dt_qk, tag="KT")
        Vp = const.tile([P, JT, 130], dt_pv, tag="Vp")
        OT0 = const.tile([HD, N], dt_pv, tag="OT0")
        OT1 = const.tile([HD, N], dt_pv, tag="OT1")
        ones = const.tile([65, HD], f32, tag="ones")
        nc.vector.memset(ones[64:65, :], 1.0)
        actwarm = const.tile([1, 1], f32, tag="actwarm")
        nc.vector.memset(actwarm[:], 0.0)
        nc.scalar.activation(actwarm[:], actwarm[:], Exp)
        nc.vector.memset(Vp[:, :, 64:65], 1.0)
        nc.vector.memset(Vp[:, :, 129:130], 1.0)

        # PE warmup during the initial DMA window: the HAM clock gate needs
        # ~3.4us of sustained PE activity to reach 2.4 GHz; these junk
        # matmuls (on a memset tile, result overwritten later) convert the
        # dead weight/x DMA wait into warmup so the first projections and
        # score matmuls run at full clock.
        # Dense PE warmup during the initial DMA window: the HAM clock gate
        # un-throttles (1.2 -> 2.4 GHz) only after a ~3.4us window of
        # near-100% PE activity, so these N=512 junk matmuls run
        # back-to-back alternating the two s-pool banks; the first real
        # projections then run at full clock.
        warm = const.tile([P, 512], dt_qk, tag="warm")
        nc.vector.memset(warm[:], 0.0)
        for i in range(8):
            jk = sp.tile([P, 1024], f32, tag="s", name=f"junk_{i}")
            nc.tensor.matmul(jk[:, 0:512], warm[:, 0:P], warm[:],
                             start=True, stop=True)

        # ---- projection units (emitted interleaved into the attention loop
        # so the PE prefix before the first exp is tiny) ----
        def qproj_unit(ic):
            # Q^T[:, ic] (uses the otherwise-idle mp psum bank)
            ps = mp.tile([P, 512], f32, tag="mp", name=f"qp_{ic}")
            for k in range(KT_TILES):
                nc.tensor.matmul(ps[:], wqt[:, k, :], xt[:, k, ics(ic)],
                                 start=(k == 0), stop=(k == KT_TILES - 1))
            nc.vector.tensor_scalar_add(QT[:, ics(ic)], ps[:], bq[:, 0:1])

        def qproj_quarter(ic, q, after=None):
            # one [128,128] N-slice of the Q^T projection (~0.5us PE quantum)
            qs = slice(ics(ic).start + 128 * q, ics(ic).start + 128 * (q + 1))
            ps = mp.tile([P, 128], f32, tag="mp", name=f"qq_{ic}_{q}")
            for k in range(KT_TILES):
                mm = nc.tensor.matmul(ps[:], wqt[:, k, :], xt[:, k, qs],
                                      start=(k == 0), stop=(k == KT_TILES - 1))
                if after is not None and k == 0:
                    _add_dep_helper(mm.ins, after.ins, sync=False,
                                    reason="defer qproj behind attention")
            nc.vector.tensor_scalar_add(QT[:, qs], ps[:], bq[:, 0:1])

        def kproj_cols(c0, c1, after=None):
            # K^T projection for columns [c0, c1) (<= 512 wide)
            qs = slice(c0, c1)
            ps = mp.tile([P, 512], f32, tag="mp", name=f"kp_{c0}")
            for k in range(KT_TILES):
                mm = nc.tensor.matmul(ps[:, 0:c1 - c0], wkt[:, k, :],
                                      xt[:, k, qs],
                                      start=(k == 0), stop=(k == KT_TILES - 1))
                if after is not None and k == 0:
                    _add_dep_helper(mm.ins, after.ins, sync=False,
                                    reason="defer kproj behind attention")
            nc.vector.tensor_scalar_add(KT[:, qs], ps[:, 0:c1 - c0], bk[:, 0:1])

        def kproj_unit(jc, after=None):
            kproj_cols(jc * 512, (jc + 1) * 512, after=after)

        def vproj_unit(jt, after=None):
            # V[jt] (natural layout) + split into the two per-head Vp slabs
            ps = op.tile([P, P], f32, tag="o", name=f"vp_{jt}")
            for k in range(KT_TILES):
                mm = nc.tensor.matmul(ps[:], xt[:, k, jts(jt)], wvt[:, k, :],
                                      start=(k == 0), stop=(k == KT_TILES - 1))
                if after is not None and k == 0:
                    _add_dep_helper(mm.ins, after.ins, sync=False,
                                    reason="defer vproj behind attention")
            nc.vector.tensor_copy(Vp[:, jt, 0:64], ps[:, 0:64])
            nc.vector.tensor_copy(Vp[:, jt, 65:129], ps[:, 64:128])

        # upfront: Q chunk 0 + the first 128 K columns only -- the minimum
        # for the first score pair, so exp #0 starts as early as possible.
        # The rest of K lands via per-iteration extras in ic=0.
        qproj_unit(0)
        kproj_cols(0, 128)

        # ---- attention (software-pipelined emission: S/exp of step t,
        # independent extras, then PV of step t-1 LAST so the exp-dependent
        # PV never blocks the next score pair in the PE FIFO).
        # finalize is staged: DVE-only work (psum evict + reciprocal) right
        # after the last PV; PE work (outer-product, OT mul, y projection)
        # several iterations later so the PE FIFO never waits on the
        # reciprocal. ----
        def finalize_a(ic, o0, o1):
            # PSUM evictions only -- frees the o banks fast; everything slow
            # happens later, off the PE critical path
            oss = []
            for i, o in enumerate((o0, o1)):
                os_ = rrp.tile([65, 512], f32, tag="os", name=f"os_{ic}_{i}")
                nc.vector.tensor_copy(os_[:], o[:])
                oss.append(os_)
            return oss

        def norm_quantum(ic, st, h, half, after=None):
            # normalize one head/half: OT[:, slice] = os[0:64] * (1/r) via PE
            # outer-product of the raw denominator (no recip dep in PE FIFO)
            # then reciprocal+mul on DVE.  ~0.5us of PE work per quantum.
            os_ = st[h]
            OTt = (OT0, OT1)[h]
            ls = slice(256 * half, 256 * (half + 1))
            hs = slice(ics(ic).start + 256 * half, ics(ic).start + 256 * (half + 1))
            pool_h = mp if h == 0 else op
            rb = pool_h.tile([HD, 256], f32, tag="mp" if h == 0 else "o",
                             name=f"rb_{ic}_{h}_{half}")
            mm = nc.tensor.matmul(rb[:], ones[64:65, :], os_[64:65, ls],
                                  start=True, stop=True, tile_position=(64, 0))
            if after is not None:
                _add_dep_helper(mm.ins, after.ins, sync=False,
                                reason="defer finalize rb behind attention")
            rbs = rbp.tile([HD, 256], f32, tag="rbs", name=f"rbs_{ic}_{h}_{half}")
            nc.vector.reciprocal_approx_fast(rbs[:], rb[:])
            nc.vector.tensor_mul(OTt[:, hs], os_[0:64, ls], rbs[:])

        def yproj_quantum(ic, mt, half, idx, after=None):
            # one [128,256] slice of the partial output projection
            hs = slice(ics(ic).start + 256 * half, ics(ic).start + 256 * (half + 1))
            pool_y = mp if idx % 2 == 0 else op
            yps = pool_y.tile([P, 256], f32, tag="mp" if idx % 2 == 0 else "o",
                              name=f"yp_{ic}_{mt}_{half}")
            mm = nc.tensor.matmul(yps[:], wot[:, 0, mts(mt)], OT0[:, hs],
                                  start=True, stop=False)
            if after is not None:
                _add_dep_helper(mm.ins, after.ins, sync=False,
                                reason="defer finalize yproj behind attention")
            nc.tensor.matmul(yps[:], wot[:, 1, mts(mt)], OT1[:, hs],
                             start=False, stop=True)
            ye = yep.tile([P, 256], f32, tag="ye", name=f"ye_{ic}_{mt}_{half}")
            nc.vector.tensor_copy(ye[:], yps[:])
            nc.sync.dma_start(yp_d[mt, :, hs], ye[:])

        # quantum schedule within the NEXT chunk: (jt, fn(args))
        NORM_SCHED = [(2, (0, 0)), (4, (1, 0)), (5, (0, 1)), (7, (1, 1))]
        YP_JT0 = 9

        otiles = {}
        pend = None          # (p_tile, ic, jt) whose PV is not yet emitted
        pend_b = None        # (ic, stage-a state) awaiting finalize_b
        for ic in range(ICH):
            otiles[ic] = (op.tile([65, 512], f32, tag="o", name=f"o0_{ic}"),
                          op.tile([65, 512], f32, tag="o", name=f"o1_{ic}"))
            for jt in range(JT):
                s = sp.tile([P, 1024], f32, tag="s")
                last_s = nc.tensor.matmul(s[:, 0:512], KT[0:64, jts(jt)],
                                          QT[0:64, ics(ic)],
                                          start=True, stop=True,
                                          tile_position=(0, 0))
                nc.tensor.matmul(s[:, 512:1024], KT[64:128, jts(jt)],
                                 QT[64:128, ics(ic)],
                                 start=True, stop=True, tile_position=(64, 0))
                p = pp.tile([P, 1024], dt_pv, tag="p")
                nc.scalar.activation(p[:], s[:], Exp)
                # ---- independent extras (proj + deferred finalize stages),
                # emitted BEFORE the exp-dependent PV pair ----
                if ic == 0:
                    # K-projection schedule: tiles jt1-3 right behind the
                    # first score pair, then one 512-wide unit every few
                    # iterations, always >= 2 iterations ahead of use
                    if jt == 0:
                        kproj_cols(128, 512, after=last_s)
                    elif jt == 1:
                        kproj_unit(1, after=last_s)
                    elif 3 <= jt < 27 and (jt - 3) % 4 == 0:
                        kproj_unit(2 + (jt - 3) // 4, after=last_s)
                    vproj_unit(jt, after=last_s)
                if pend_b is not None:
                    bic, st = pend_b
                    for sjt, (h, half) in NORM_SCHED:
                        if jt == sjt:
                            norm_quantum(bic, st, h, half, after=last_s)
                    if YP_JT0 <= jt < YP_JT0 + 8:
                        idx = jt - YP_JT0        # A halves then B halves
                        half, mt = divmod(idx, 4)
                        yproj_quantum(bic, mt, half, idx, after=last_s)
                        if idx == 7:
                            pend_b = None
                if 24 <= jt < 28 and ic + 1 < ICH:
                    qproj_quarter(ic + 1, jt - 24, after=last_s)
                # ---- PV of the previous step (depends on its exp) LAST ----
                if pend is not None:
                    pp_, pic, pjt = pend
                    o0, o1 = otiles[pic]
                    nc.tensor.matmul(o0[:], Vp[:, pjt, 0:65], pp_[:, 0:512],
                                     start=(pjt == 0), stop=(pjt == JT - 1))
                    nc.tensor.matmul(o1[:], Vp[:, pjt, 65:130], pp_[:, 512:1024],
                                     start=(pjt == 0), stop=(pjt == JT - 1))
                    if pjt == JT - 1:
                        pend_b = (pic, finalize_a(pic, o0, o1))
                pend = (p, ic, jt)
        # ---- drain the pipeline tail (no exp left to overlap: use N=512
        # quanta, approx reciprocal, and spread evictions/DMAs over the idle
        # Scalar/Vector engines and all four DMA queues) ----
        pp_, pic, pjt = pend
        o0, o1 = otiles[pic]
        nc.tensor.matmul(o0[:], Vp[:, pjt, 0:65], pp_[:, 0:512],
                         start=(pjt == 0), stop=(pjt == JT - 1))
        nc.tensor.matmul(o1[:], Vp[:, pjt, 65:130], pp_[:, 512:1024],
                         start=(pjt == 0), stop=(pjt == JT - 1))
        oss = []
        for i, o in enumerate((o0, o1)):
            os_ = rrp.tile([65, 512], f32, tag="os", name=f"ost_{i}")
            nc.scalar.copy(os_[:], o[:])       # ScalarE is idle in the tail
            oss.append(os_)
        rbts = []
        for h in (0, 1):
            pool_h = mp if h == 0 else op
            rb = pool_h.tile([HD, 512], f32, tag="mp" if h == 0 else "o",
                             name=f"rbt_{h}")
            nc.tensor.matmul(rb[:], ones[64:65, :], oss[h][64:65, :],
                             start=True, stop=True, tile_position=(64, 0))
            rbts.append(rb)
        # keep the PE HAM-warm through the finalize DVE chain (reciprocal +
        # normalize) so the tail y-projection matmuls run at 2.4 GHz, not
        # re-throttled 1.2
        for i in range(6):
            jk = sp.tile([P, 1024], f32, tag="s", name=f"tjunk_{i}")
            nc.tensor.matmul(jk[:, 0:512], warm[:, 0:P], warm[:],
                             start=True, stop=True)
        for h in (0, 1):
            OTt = (OT0, OT1)[h]
            rbs = rbp.tile([HD, 512], f32, tag="rbs", name=f"rbst_{h}")
            nc.vector.reciprocal_approx_fast(rbs[:], rbts[h][:])
            nc.vector.tensor_mul(OTt[:, ics(pic)], oss[h][0:64, :], rbs[:])
        for mt in range(4):
            pool_y = mp if mt % 2 == 0 else op
            yps = pool_y.tile([P, 512], f32, tag="mp" if mt % 2 == 0 else "o",
                              name=f"ypt_{mt}")
            nc.tensor.matmul(yps[:], wot[:, 0, mts(mt)], OT0[:, ics(pic)],
                             start=True, stop=False)
            nc.tensor.matmul(yps[:], wot[:, 1, mts(mt)], OT1[:, ics(pic)],
                             start=False, stop=True)
            ye = yep.tile([P, 512], f32, tag="ye", name=f"yet_{mt}")
            nc.scalar.copy(ye[:], yps[:])     # ScalarE idle; DVE queue lags
            q = (nc.sync, nc.sync, nc.sync, nc.sync)[mt]
            q.dma_start(yp_d[mt, :, ics(pic)], ye[:])

    nc.compile()
    _BUILD_CACHE[mode] = nc
    return nc


def _prep_inputs(x, qkv_w, qkv_b, out_w, mode):
    """Per-core input maps. Core c: batch c//4, head-pair c%4."""
    if mode == "bf16":
        dt_qk = np.dtype(ml_dtypes.bfloat16)
        dt_pv = dt_qk
    elif mode == "mixed":
        dt_qk = np.dtype(np.float32)
        dt_pv = np.dtype(ml_dtypes.bfloat16)
    else:
        dt_qk = np.dtype(np.float32)
        dt_pv = dt_qk

    x = np.asarray(x, np.float32)
    qkv_w = np.asarray(qkv_w, np.float32)
    qkv_b = np.asarray(qkv_b, np.float32)
    out_w = np.asarray(out_w, np.float32)

    xts = []
    for b in range(B):
        xt = np.ascontiguousarray(x[b].T).reshape(KT_TILES, P, N)
        xts.append(xt.astype(dt_qk))

    in_maps = []
    for c in range(NCORES):
        b, m = divmod(c, 4)
        rs = slice(P * m, P * (m + 1))
        wq = (0.125 * qkv_w[0:D][rs]).T.reshape(KT_TILES, P, P)
        wk = qkv_w[D:2 * D][rs].T.reshape(KT_TILES, P, P)
        wv = qkv_w[2 * D:3 * D][rs].T.reshape(KT_TILES, P, P)
        wo = np.ascontiguousarray(out_w[:, rs].T).reshape(2, HD, D)
        in_maps.append({
            "xt": xts[b],
            "wqt": np.ascontiguousarray(wq).astype(dt_qk),
            "wkt": np.ascontiguousarray(wk).astype(dt_qk),
            "wvt": np.ascontiguousarray(wv).astype(dt_qk),
            "wot": wo.astype(dt_pv),
            "bq": (0.125 * qkv_b[0:D][rs]).reshape(P, 1).astype(np.float32),
            "bk": qkv_b[D:2 * D][rs].reshape(P, 1).astype(np.float32),
        })
    return in_maps


def _gather(results, qkv_b, out_w, out_b):
    # y[b] = (sum over the batch's 4 cores of yp)^T + out_w @ bv + out_b
    bias_vec = out_w.astype(np.float32) @ np.asarray(qkv_b, np.float32)[2 * D:3 * D] \
        + np.asarray(out_b, np.float32)
    y = np.empty((B, N, D), np.float32)
    for b in range(B):
        acc = np.zeros((D, N), np.float32)
        for m in range(4):
            acc += results[4 * b + m]["yp"].reshape(D, N)
        y[b] = acc.T + bias_vec
    return y


def _run(inputs, trace=False, tmpdir=None):
    from concourse.bass_utils import run_bass_kernel_spmd

    nc = _build(MODE)
    in_maps = _prep_inputs(inputs["x"], inputs["qkv_w"], inputs["qkv_b"],
                           inputs["out_w"], MODE)
    kw = {}
    if trace:
        kw = dict(trace=True, tmpdir=tmpdir)
    res = run_bass_kernel_spmd(nc, in_maps, core_ids=list(range(NCORES)), **kw)
    y = _gather(res.results, inputs["qkv_b"], inputs["out_w"], inputs["out_b"])
    return y, res


def kernel(x, qkv_w, qkv_b, out_w, out_b):
    y, _ = _run(dict(x=x, qkv_w=qkv_w, qkv_b=qkv_b, out_w=out_w, out_b=out_b))
    return y


# revision 33
# speedup vs baseline: 1.0225x; 1.0225x over previous
"""Multi-head self-attention Trainium2 kernel (8 NeuronCores, SPMD).

Problem: B=2, N=4096, D=512, H=8 heads of dim 64.
  qkv = x @ qkv_w.T + qkv_b ; per-head attention with softmax(QK^T/8) ;
  out = attn @ out_w.T + out_b

Sharding: 16 (batch, head) pairs -> 8 cores, each core owns one batch b and
one head-PAIR (2 adjacent heads = a 128-row slice of the qkv projections).
Each core computes the full attention for its 2 heads over all 4096 rows and
a partial output projection; the host sums the 4 per-batch partials and adds
the (folded) biases.

On-chip layout strategy: everything is computed with the contraction dim on
partitions so no transposes are ever needed:
  Q^T,K^T [128d, 4096]  <- lhsT=W^T tiles, rhs=x^T
  V       [4096, 128d]  (natural; lhsT=x^T tile, rhs=Wv^T) + fused ones column
  S^T = K^T-stationary matmul, 2 heads row-packed (K=64 each) in the PE array
  P^T = exp(S^T) on ScalarE straight out of PSUM (no max-subtraction: |S|<~3)
  O^T accum = (V|1)-stationary matmul over P^T; row 64 = softmax denominator
  normalize via reciprocal + PE outer-product broadcast; partial y^T = Wout^T
  slice-stationary matmul.
Scale 1/sqrt(64) and all biases are folded on the host (wq*=0.125 etc.).

The steady state is ScalarE-exp-saturated (256 x [128,1024] Exp at ~1.11us
back-to-back = 285us); the schedule exists to keep that chain gapless:
  - PE warmup junk matmuls during the initial weight/x DMA window (HAM
    un-throttle) + minimal K projection (one 128-col quarter) before the
    first score pair, so exp #0 starts ~7us earlier.
  - K/V projections are flattened to one quantum per iteration across ic=0
    (kproj quarter at iter jt covers the jt+1 key tile just in time).
  - Per iteration the emission order is [score-pair; independent extras;
    PV(prev)], so the exp-dependent PV pair never head-of-line-blocks the
    next score pair in the PE FIFO.
  - finalize: reciprocal_approx_fast (~5x vs iterative reciprocal), and the
    pipeline-drain tail uses N=512 quanta with evictions/DMAs spread across
    the otherwise-idle Scalar/Vector engines and all four DMA queues.
"""

import os
import numpy as np
import ml_dtypes

B, N, D, H, HD = 2, 4096, 512, 8, 64
NCORES = 8
KT_TILES = 4      # D / 128 contraction tiles
JT = 32           # N / 128 key tiles
ICH = 8           # N / 512 query chunks
P = 128

# compute dtype mode: "bf16" (fast), "mixed" (fp32 scores, bf16 PV),
# "fp32" (all fp32)
MODE = os.environ.get("ATTN_KERNEL_MODE", "bf16")

_BUILD_CACHE = {}


def _np_dt(dt):
    import concourse.mybir as mybir
    return np.dtype(ml_dtypes.bfloat16) if dt == mybir.dt.bfloat16 else np.dtype(np.float32)


def _build(mode):
    """Build (and cache) the compiled Bass program for all cores (SPMD)."""
    if mode in _BUILD_CACHE:
        return _BUILD_CACHE[mode]

    import concourse.bacc as bacc
    import concourse.mybir as mybir
    import concourse.tile as tile
    from concourse.bass import _add_dep_helper
    from contextlib import ExitStack

    f32 = mybir.dt.float32
    bf16 = mybir.dt.bfloat16
    if mode == "bf16":
        dt_qk, dt_pv = bf16, bf16
    elif mode == "mixed":
        dt_qk, dt_pv = f32, bf16
    else:
        dt_qk, dt_pv = f32, f32

    Exp = mybir.ActivationFunctionType.Exp

    nc = bacc.Bacc(None, target_bir_lowering=False)
    xt_d = nc.dram_tensor("xt", [KT_TILES, P, N], dt_qk, kind="ExternalInput")
    wqt_d = nc.dram_tensor("wqt", [KT_TILES, P, P], dt_qk, kind="ExternalInput")
    wkt_d = nc.dram_tensor("wkt", [KT_TILES, P, P], dt_qk, kind="ExternalInput")
    wvt_d = nc.dram_tensor("wvt", [KT_TILES, P, P], dt_qk, kind="ExternalInput")
    wot_d = nc.dram_tensor("wot", [2, HD, D], dt_pv, kind="ExternalInput")
    bq_d = nc.dram_tensor("bq", [P, 1], f32, kind="ExternalInput")
    bk_d = nc.dram_tensor("bk", [P, 1], f32, kind="ExternalInput")
    yp_d = nc.dram_tensor("yp", [KT_TILES, P, N], f32, kind="ExternalOutput")

    def ics(i):
        return slice(i * 512, (i + 1) * 512)

    def jts(j):
        return slice(j * P, (j + 1) * P)

    def mts(m):
        return slice(m * P, (m + 1) * P)

    with tile.TileContext(nc) as tc, ExitStack() as ctx:
        const = ctx.enter_context(tc.tile_pool(name="const", bufs=1))
        sp = ctx.enter_context(tc.tile_pool(name="spool", bufs=2, space="PSUM"))
        op = ctx.enter_context(tc.tile_pool(name="opool", bufs=3, space="PSUM"))
        mp = ctx.enter_context(tc.tile_pool(name="mpool", bufs=1, space="PSUM"))
        pp = ctx.enter_context(tc.tile_pool(name="ppool", bufs=6))
        yep = ctx.enter_context(tc.tile_pool(name="yepool", bufs=3))
        rrp = ctx.enter_context(tc.tile_pool(name="rrpool", bufs=2))
        rbp = ctx.enter_context(tc.tile_pool(name="rbpool", bufs=2))

        xt = const.tile([P, KT_TILES, N], dt_qk, tag="xt")
        wqt = const.tile([P, KT_TILES, P], dt_qk, tag="wqt")
        wkt = const.tile([P, KT_TILES, P], dt_qk, tag="wkt")
        wvt = const.tile([P, KT_TILES, P], dt_qk, tag="wvt")
        # Queue assignment tuned for the prologue critical chain
        # (qproj -> kproj -> first score pair -> first exp):
        #  - sync (fast HW queue): wqt interleaved with x^T chunk 0 so the
        #    Q projection's weights AND rhs land first, then the remaining
        #    x chunks in column-major order (early kproj/vproj can start
        #    while the bulk of x streams in).
        #  - gpsimd (slow SW queue): only wkt (needed ~4us in).
        #  - scalar (shares the ScalarE FIFO with the exps -- keep short):
        #    bq/bk first (bias adds gate the first score pair), then the
        #    late-needed wvt/wot.
        bq = const.tile([P, 1], f32, tag="bq")
        bk = const.tile([P, 1], f32, tag="bk")
        for k in range(KT_TILES):
            nc.sync.dma_start(wqt[:, k, :], wqt_d[k])
        for k in range(KT_TILES):
            nc.gpsimd.dma_start(wkt[:, k, :], wkt_d[k])
        for k in range(KT_TILES):
            nc.scalar.dma_start(wvt[:, k, :], wvt_d[k])
        nc.gpsimd.dma_start(bq[:], bq_d[:])
        nc.gpsimd.dma_start(bk[:], bk_d[:])
        # x^T in column-major chunk order: the first column blocks (all
        # k-tiles) land right after wqt, so the Q/K projections and early
        # attention start ~7us sooner than waiting for whole k-tiles
        XCH = 512
        for c in range(N // XCH):
            for k in range(KT_TILES):
                nc.sync.dma_start(xt[:, k, c * XCH:(c + 1) * XCH],
                                  xt_d[k][:, c * XCH:(c + 1) * XCH])
        wot = const.tile([HD, 2, D], dt_pv, tag="wot")
        for h in range(2):
            nc.scalar.dma_start(wot[:, h, :], wot_d[h])

        QT = const.tile([P, N], dt_qk, tag="QT")
        KT = const.tile([P, N], dt_qk, tag="KT")
        Vp = const.tile([P, JT, 130], dt_pv, tag="Vp")
        OT0 = const.tile([HD, N], dt_pv, tag="OT0")
        OT1 = const.tile([HD, N], dt_pv, tag="OT1")
        ones = const.tile([65, HD], f32, tag="ones")
        nc.vector.memset(ones[64:65, :], 1.0)
        actwarm = const.tile([1, 1], f32, tag="actwarm")
        nc.vector.memset(actwarm[:], 0.0)
        nc.scalar.activation(actwarm[:], actwarm[:], Exp)
        nc.vector.memset(Vp[:, :, 64:65], 1.0)
        nc.vector.memset(Vp[:, :, 129:130], 1.0)

        # PE warmup during the initial DMA window: the HAM clock gate needs
        # ~3.4us of sustained PE activity to reach 2.4 GHz; these junk
        # matmuls (on a memset tile, result overwritten later) convert the
        # dead weight/x DMA wait into warmup so the first projections and
        # score matmuls run at full clock.
        # Dense PE warmup during the initial DMA window: the HAM clock gate
        # un-throttles (1.2 -> 2.4 GHz) only after a ~3.4us window of
        # near-100% PE activity, so these N=512 junk matmuls run
        # back-to-back alternating the two s-pool banks; the first real
        # projections then run at full clock.
        warm = const.tile([P, 512], dt_qk, tag="warm")
        nc.vector.memset(warm[:], 0.0)
        for i in range(8):
            jk = sp.tile([P, 1024], f32, tag="s", name=f"junk_{i}")
            nc.tensor.matmul(jk[:, 0:512], warm[:, 0:P], warm[:],
                             start=True, stop=True)

        # ---- projection units (emitted interleaved into the attention loop
        # so the PE prefix before the first exp is tiny) ----
        def qproj_unit(ic):
            # Q^T[:, ic] (uses the otherwise-idle mp psum bank)
            ps = mp.tile([P, 512], f32, tag="mp", name=f"qp_{ic}")
            for k in range(KT_TILES):
                nc.tensor.matmul(ps[:], wqt[:, k, :], xt[:, k, ics(ic)],
                                 start=(k == 0), stop=(k == KT_TILES - 1))
            nc.vector.tensor_scalar_add(QT[:, ics(ic)], ps[:], bq[:, 0:1])

        def qproj_quarter(ic, q, after=None):
            # one [128,128] N-slice of the Q^T projection (~0.5us PE quantum)
            qs = slice(ics(ic).start + 128 * q, ics(ic).start + 128 * (q + 1))
            ps = mp.tile([P, 128], f32, tag="mp", name=f"qq_{ic}_{q}")
            for k in range(KT_TILES):
                mm = nc.tensor.matmul(ps[:], wqt[:, k, :], xt[:, k, qs],
                                      start=(k == 0), stop=(k == KT_TILES - 1))
                if after is not None and k == 0:
                    _add_dep_helper(mm.ins, after.ins, sync=False,
                                    reason="defer qproj behind attention")
            nc.vector.tensor_scalar_add(QT[:, qs], ps[:], bq[:, 0:1])

        def kproj_cols(c0, c1, after=None):
            # K^T projection for columns [c0, c1) (<= 512 wide)
            qs = slice(c0, c1)
            ps = mp.tile([P, 512], f32, tag="mp", name=f"kp_{c0}")
            for k in range(KT_TILES):
                mm = nc.tensor.matmul(ps[:, 0:c1 - c0], wkt[:, k, :],
                                      xt[:, k, qs],
                                      start=(k == 0), stop=(k == KT_TILES - 1))
                if after is not None and k == 0:
                    _add_dep_helper(mm.ins, after.ins, sync=False,
                                    reason="defer kproj behind attention")
            nc.vector.tensor_scalar_add(KT[:, qs], ps[:, 0:c1 - c0], bk[:, 0:1])

        def kproj_unit(jc, after=None):
            kproj_cols(jc * 512, (jc + 1) * 512, after=after)

        def kproj_unit_s(jc):
            # K^T chunk on an s-pool slot (prefix only: runs parallel to the
            # qproj on the mp bank)
            ps = sp.tile([P, 1024], f32, tag="s", name=f"kps_{jc}")
            for k in range(KT_TILES):
                nc.tensor.matmul(ps[:, 0:512], wkt[:, k, :], xt[:, k, ics(jc)],
                                 start=(k == 0), stop=(k == KT_TILES - 1))
            nc.vector.tensor_scalar_add(KT[:, ics(jc)], ps[:, 0:512], bk[:, 0:1])

        def vproj_unit(jt, after=None):
            # V[jt] (natural layout) + split into the two per-head Vp slabs
            ps = op.tile([P, P], f32, tag="o", name=f"vp_{jt}")
            for k in range(KT_TILES):
                mm = nc.tensor.matmul(ps[:], xt[:, k, jts(jt)], wvt[:, k, :],
                                      start=(k == 0), stop=(k == KT_TILES - 1))
                if after is not None and k == 0:
                    _add_dep_helper(mm.ins, after.ins, sync=False,
                                    reason="defer vproj behind attention")
            nc.vector.tensor_copy(Vp[:, jt, 0:64], ps[:, 0:64])
            nc.vector.tensor_copy(Vp[:, jt, 65:129], ps[:, 64:128])

        # upfront: Q chunk 0 on mp, K chunks 0+1 on the two s-pool slots
        qproj_unit(0)
        kproj_unit_s(0)
        kproj_unit_s(1)

        # ---- attention (software-pipelined emission: S/exp of step t,
        # independent extras, then PV of step t-1 LAST so the exp-dependent
        # PV never blocks the next score pair in the PE FIFO).
        # finalize is staged: DVE-only work (psum evict + reciprocal) right
        # after the last PV; PE work (outer-product, OT mul, y projection)
        # several iterations later so the PE FIFO never waits on the
        # reciprocal. ----
        def finalize_a(ic, o0, o1):
            # PSUM evictions only -- frees the o banks fast; everything slow
            # happens later, off the PE critical path
            oss = []
            for i, o in enumerate((o0, o1)):
                os_ = rrp.tile([65, 512], f32, tag="os", name=f"os_{ic}_{i}")
                nc.vector.tensor_copy(os_[:], o[:])
                oss.append(os_)
            return oss

        def norm_quantum(ic, st, h, half, after=None):
            # normalize one head/half: OT[:, slice] = os[0:64] * (1/r) via PE
            # outer-product of the raw denominator (no recip dep in PE FIFO)
            # then reciprocal+mul on DVE.  ~0.5us of PE work per quantum.
            os_ = st[h]
            OTt = (OT0, OT1)[h]
            ls = slice(256 * half, 256 * (half + 1))
            hs = slice(ics(ic).start + 256 * half, ics(ic).start + 256 * (half + 1))
            pool_h = mp if h == 0 else op
            rb = pool_h.tile([HD, 256], f32, tag="mp" if h == 0 else "o",
                             name=f"rb_{ic}_{h}_{half}")
            mm = nc.tensor.matmul(rb[:], ones[64:65, :], os_[64:65, ls],
                                  start=True, stop=True, tile_position=(64, 0))
            if after is not None:
                _add_dep_helper(mm.ins, after.ins, sync=False,
                                reason="defer finalize rb behind attention")
            rbs = rbp.tile([HD, 256], f32, tag="rbs", name=f"rbs_{ic}_{h}_{half}")
            nc.vector.reciprocal_approx_fast(rbs[:], rb[:])
            nc.vector.tensor_mul(OTt[:, hs], os_[0:64, ls], rbs[:])

        def yproj_quantum(ic, mt, half, idx, after=None):
            # one [128,256] slice of the partial output projection
            hs = slice(ics(ic).start + 256 * half, ics(ic).start + 256 * (half + 1))
            pool_y = mp if idx % 2 == 0 else op
            yps = pool_y.tile([P, 256], f32, tag="mp" if idx % 2 == 0 else "o",
                              name=f"yp_{ic}_{mt}_{half}")
            mm = nc.tensor.matmul(yps[:], wot[:, 0, mts(mt)], OT0[:, hs],
                                  start=True, stop=False)
            if after is not None:
                _add_dep_helper(mm.ins, after.ins, sync=False,
                                reason="defer finalize yproj behind attention")
            nc.tensor.matmul(yps[:], wot[:, 1, mts(mt)], OT1[:, hs],
                             start=False, stop=True)
            ye = yep.tile([P, 256], f32, tag="ye", name=f"ye_{ic}_{mt}_{half}")
            nc.vector.tensor_copy(ye[:], yps[:])
            nc.sync.dma_start(yp_d[mt, :, hs], ye[:])

        # quantum schedule within the NEXT chunk: (jt, fn(args))
        NORM_SCHED = [(2, (0, 0)), (4, (1, 0)), (5, (0, 1)), (7, (1, 1))]
        YP_JT0 = 9

        otiles = {}
        pend = None          # (p_tile, ic, jt) whose PV is not yet emitted
        pend_b = None        # (ic, stage-a state) awaiting finalize_b
        for ic in range(ICH):
            otiles[ic] = (op.tile([65, 512], f32, tag="o", name=f"o0_{ic}"),
                          op.tile([65, 512], f32, tag="o", name=f"o1_{ic}"))
            for jt in range(JT):
                s = sp.tile([P, 1024], f32, tag="s")
                last_s = nc.tensor.matmul(s[:, 0:512], KT[0:64, jts(jt)],
                                          QT[0:64, ics(ic)],
                                          start=True, stop=True,
                                          tile_position=(0, 0))
                nc.tensor.matmul(s[:, 512:1024], KT[64:128, jts(jt)],
                                 QT[64:128, ics(ic)],
                                 start=True, stop=True, tile_position=(64, 0))
                p = pp.tile([P, 1024], dt_pv, tag="p")
                nc.scalar.activation(p[:], s[:], Exp)
                # ---- independent extras (proj + deferred finalize stages),
                # emitted BEFORE the exp-dependent PV pair ----
                if ic == 0:
                    if jt == 0:
                        vproj_unit(0, after=last_s)
                        vproj_unit(1, after=last_s)
                    elif jt <= JT - 2:
                        vproj_unit(jt + 1, after=last_s)
                    if jt < 24 and jt % 4 == 0:
                        kproj_unit(2 + jt // 4, after=last_s)
                if pend_b is not None:
                    bic, st = pend_b
                    for sjt, (h, half) in NORM_SCHED:
                        if jt == sjt:
                            norm_quantum(bic, st, h, half, after=last_s)
                    if YP_JT0 <= jt < YP_JT0 + 8:
                        idx = jt - YP_JT0        # A halves then B halves
                        half, mt = divmod(idx, 4)
                        yproj_quantum(bic, mt, half, idx, after=last_s)
                        if idx == 7:
                            pend_b = None
                if 24 <= jt < 28 and ic + 1 < ICH:
                    qproj_quarter(ic + 1, jt - 24, after=last_s)
                # ---- PV of the previous step (depends on its exp) LAST ----
                if pend is not None:
                    pp_, pic, pjt = pend
                    o0, o1 = otiles[pic]
                    nc.tensor.matmul(o0[:], Vp[:, pjt, 0:65], pp_[:, 0:512],
                                     start=(pjt == 0), stop=(pjt == JT - 1))
                    nc.tensor.matmul(o1[:], Vp[:, pjt, 65:130], pp_[:, 512:1024],
                                     start=(pjt == 0), stop=(pjt == JT - 1))
                    if pjt == JT - 1:
                        pend_b = (pic, finalize_a(pic, o0, o1))
                pend = (p, ic, jt)
        # ---- drain the pipeline tail (no exp left to overlap: use N=512
        # quanta, approx reciprocal, and spread evictions/DMAs over the idle
        # Scalar/Vector engines and all four DMA queues) ----
        pp_, pic, pjt = pend
        o0, o1 = otiles[pic]
        nc.tensor.matmul(o0[:], Vp[:, pjt, 0:65], pp_[:, 0:512],
                         start=(pjt == 0), stop=(pjt == JT - 1))
        nc.tensor.matmul(o1[:], Vp[:, pjt, 65:130], pp_[:, 512:1024],
                         start=(pjt == 0), stop=(pjt == JT - 1))
        oss = []
        for i, o in enumerate((o0, o1)):
            os_ = rrp.tile([65, 512], f32, tag="os", name=f"ost_{i}")
            nc.scalar.copy(os_[:], o[:])       # ScalarE is idle in the tail
            oss.append(os_)
        rbts = []
        for h in (0, 1):
            pool_h = mp if h == 0 else op
            rb = pool_h.tile([HD, 512], f32, tag="mp" if h == 0 else "o",
                             name=f"rbt_{h}")
            nc.tensor.matmul(rb[:], ones[64:65, :], oss[h][64:65, :],
                             start=True, stop=True, tile_position=(64, 0))
            rbts.append(rb)
        # keep the PE HAM-warm through the finalize DVE chain (reciprocal +
        # normalize) so the tail y-projection matmuls run at 2.4 GHz, not
        # re-throttled 1.2
        for i in range(6):
            jk = sp.tile([P, 1024], f32, tag="s", name=f"tjunk_{i}")
            nc.tensor.matmul(jk[:, 0:512], warm[:, 0:P], warm[:],
                             start=True, stop=True)
        for h in (0, 1):
            OTt = (OT0, OT1)[h]
            rbs = rbp.tile([HD, 512], f32, tag="rbs", name=f"rbst_{h}")
            nc.vector.reciprocal_approx_fast(rbs[:], rbts[h][:])
            nc.vector.tensor_mul(OTt[:, ics(pic)], oss[h][0:64, :], rbs[:])
        for mt in range(4):
            pool_y = mp if mt % 2 == 0 else op
            yps = pool_y.tile([P, 512], f32, tag="mp" if mt % 2 == 0 else "o",
                              name=f"ypt_{mt}")
            nc.tensor.matmul(yps[:], wot[:, 0, mts(mt)], OT0[:, ics(pic)],
                             start=True, stop=False)
            nc.tensor.matmul(yps[:], wot[:, 1, mts(mt)], OT1[:, ics(pic)],
                             start=False, stop=True)
            ye = yep.tile([P, 512], f32, tag="ye", name=f"yet_{mt}")
            nc.scalar.copy(ye[:], yps[:])     # ScalarE idle; DVE queue lags
            q = (nc.sync, nc.sync, nc.sync, nc.sync)[mt]
            q.dma_start(yp_d[mt, :, ics(pic)], ye[:])

    nc.compile()
    _BUILD_CACHE[mode] = nc
    return nc


def _prep_inputs(x, qkv_w, qkv_b, out_w, mode):
    """Per-core input maps. Core c: batch c//4, head-pair c%4."""
    if mode == "bf16":
        dt_qk = np.dtype(ml_dtypes.bfloat16)
        dt_pv = dt_qk
    elif mode == "mixed":
        dt_qk = np.dtype(np.float32)
        dt_pv = np.dtype(ml_dtypes.bfloat16)
    else:
        dt_qk = np.dtype(np.float32)
        dt_pv = dt_qk

    x = np.asarray(x, np.float32)
    qkv_w = np.asarray(qkv_w, np.float32)
    qkv_b = np.asarray(qkv_b, np.float32)
    out_w = np.asarray(out_w, np.float32)

    xts = []
    for b in range(B):
        xt = np.ascontiguousarray(x[b].T).reshape(KT_TILES, P, N)
        xts.append(xt.astype(dt_qk))

    in_maps = []
    for c in range(NCORES):
        b, m = divmod(c, 4)
        rs = slice(P * m, P * (m + 1))
        wq = (0.125 * qkv_w[0:D][rs]).T.reshape(KT_TILES, P, P)
        wk = qkv_w[D:2 * D][rs].T.reshape(KT_TILES, P, P)
        wv = qkv_w[2 * D:3 * D][rs].T.reshape(KT_TILES, P, P)
        wo = np.ascontiguousarray(out_w[:, rs].T).reshape(2, HD, D)
        in_maps.append({
            "xt": xts[b],
            "wqt": np.ascontiguousarray(wq).astype(dt_qk),
            "wkt": np.ascontiguousarray(wk).astype(dt_qk),
            "wvt": np.ascontiguousarray(wv).astype(dt_qk),
            "wot": wo.astype(dt_pv),
            "bq": (0.125 * qkv_b[0:D][rs]).reshape(P, 1).astype(np.float32),
            "bk": qkv_b[D:2 * D][rs].reshape(P, 1).astype(np.float32),
        })
    return in_maps


def _gather(results, qkv_b, out_w, out_b):
    # y[b] = (sum over the batch's 4 cores of yp)^T + out_w @ bv + out_b
    bias_vec = out_w.astype(np.float32) @ np.asarray(qkv_b, np.float32)[2 * D:3 * D] \
        + np.asarray(out_b, np.float32)
    y = np.empty((B, N, D), np.float32)
    for b in range(B):
        acc = np.zeros((D, N), np.float32)
        for m in range(4):
            acc += results[4 * b + m]["yp"].reshape(D, N)
        y[b] = acc.T + bias_vec
    return y


def _run(inputs, trace=False, tmpdir=None):
    from concourse.bass_utils import run_bass_kernel_spmd

    nc = _build(MODE)
    in_maps = _prep_inputs(inputs["x"], inputs["qkv_w"], inputs["qkv_b"],
                           inputs["out_w"], MODE)
    kw = {}
    if trace:
        kw = dict(trace=True, tmpdir=tmpdir)
    res = run_bass_kernel_spmd(nc, in_maps, core_ids=list(range(NCORES)), **kw)
    y = _gather(res.results, inputs["qkv_b"], inputs["out_w"], inputs["out_b"])
    return y, res


def kernel(x, qkv_w, qkv_b, out_w, out_b):
    y, _ = _run(dict(x=x, qkv_w=qkv_w, qkv_b=qkv_b, out_w=out_w, out_b=out_b))
    return y
